# revision 1
# baseline (speedup 1.0000x reference)
"""Trainium2 Bass kernel: custom inverse STFT (degenerate per-bin rotation +
Hann window + overlap-add + window correction).

Math (matching the reference):
    F[i,k]  = S_real[i,k]*A[k] + S_imag[i,k]*B[k]
      A[k]  = w[k]*(cos(th)-sin(th))/n,  B[k] = -w[k]*(cos(th)+sin(th))/n
    out[t]  = sum_i F[i, t-256*i] / max(corr[t], 1e-8)

Sharding: 8192 frames -> 8 cores x 1024 frames.  Core m owns output blocks
[1024m, 1024m+1024) of 256 samples; it loads 3 extra "halo" frames on the
left so every owned block has all 4 overlapping contributions.  The global
tail (blocks 8192..8194, 768 samples) is reconstructed host-side from the
last 3 frames.

On-chip layout per core: frames interleaved as f = 8p + e (partition p gets 8
consecutive frames, 32KB contiguous DRAM per partition).  Overlap-add is then
free-dim-shifted adds on the DVE.  The per-partition wraparound (blocks whose
contributing frame lives on partition p+1) is produced by a shift-by-one-
partition matmul on the TensorEngine (lhsT = shifted identity) accumulating
the host-computed tail frames for partition 127 via a one-hot K=1 matmul.

Engine budget per core: DVE does Sr*A and the final add + overlap-add;
GPSIMD does Si*B (runs concurrently - fp32 1x DVE ops use its dedicated SBUF
port pair); ACT does the chunk-3 copies; PE does the halo shift.
"""

import numpy as np

import concourse.bass as bass
import concourse.bacc as bacc
import concourse.mybir as mybir
import concourse.tile as tile
from concourse.bass_utils import run_bass_kernel_spmd

F32 = mybir.dt.float32
ALU = mybir.AluOpType

P = 128            # SBUF partitions
G = 8              # frames per partition
FL = 1024          # frame length (== fft length)
FS = 256           # frame step
NF = 8192          # total frames
NCORES = 8
FPC = NF // NCORES          # frames owned per core
ROWS = FPC + 3              # input rows per core (3 left-halo frames)
OUT_LEN = FS * (NF - 1) + FL


def _window32():
    # bit-matches the reference's f32 window computation (cancellation in
    # 0.5-0.5*cos makes the f32 rounding of cos visible at the edges, and the
    # output divides by the overlap-added window — numerator and denominator
    # must use the SAME w values for the edge samples to come out right)
    k = np.arange(FL, dtype=np.float32)
    th = np.float32(2.0 * np.pi) * k / np.float32(FL)
    return (np.float32(0.5) - np.float32(0.5) * np.cos(th)).astype(np.float32)


def _coeffs():
    k = np.arange(FL, dtype=np.float64)
    th = 2.0 * np.pi * k / FL
    w = _window32().astype(np.float64)
    a = (w * (np.cos(th) - np.sin(th)) / FL).astype(np.float32)
    b = (-w * (np.cos(th) + np.sin(th)) / FL).astype(np.float32)
    return a, b


def _window_correction():
    w = _window32()
    corr = np.zeros(OUT_LEN, dtype=np.float32)
    for j in range(4):
        chunk = w[j * FS:(j + 1) * FS]
        view = corr[j * FS:j * FS + NF * FS].reshape(NF, FS)
        view += chunk[None, :]
    return corr


def _shift_weights():
    # [129, 128]: rows 0..127 = shifted identity (w[p, q] = 1 iff p == q+1),
    # row 128 = one-hot selecting output partition 127 (for the tail K=1
    # accumulation matmul)
    w = np.zeros((P + 1, P), dtype=np.float32)
    w[1:P, np.arange(P - 1)] = 0.0  # placeholder, set below
    for q in range(P - 1):
        w[q + 1, q] = 1.0
    w[P, P - 1] = 1.0
    return w


def build_nc():
    nc = bacc.Bacc(trn_type="TRN2", target_bir_lowering=False, debug=False)
    sr_d = nc.dram_tensor("s_real", [ROWS, FL], F32, kind="ExternalInput").ap()
    si_d = nc.dram_tensor("s_imag", [ROWS, FL], F32, kind="ExternalInput").ap()
    ca_d = nc.dram_tensor("coef_a", [FL], F32, kind="ExternalInput").ap()
    cb_d = nc.dram_tensor("coef_b", [FL], F32, kind="ExternalInput").ap()
    ft_d = nc.dram_tensor("f_tail", [3 * FL], F32, kind="ExternalInput").ap()
    sw_d = nc.dram_tensor("shiftw", [(P + 1) * P], F32, kind="ExternalInput").ap()
    out_d = nc.dram_tensor("out_seg", [FPC * FS], F32, kind="ExternalOutput").ap()

    # [128, 8, 1024] views: partition p holds input rows 8p..8p+7
    sr3 = sr_d[0:P * G, :].rearrange("(p g) k -> p g k", p=P)
    si3 = si_d[0:P * G, :].rearrange("(p g) k -> p g k", p=P)
    sw2 = sw_d.rearrange("(p q) -> p q", p=P + 1)
    out2 = out_d.rearrange("(p x) -> p x", p=P)      # [128, 2048]

    with tile.TileContext(nc) as tc:
        with (
            tc.tile_pool(name="const", bufs=1) as cpool,
            tc.tile_pool(name="main", bufs=1) as mpool,
            tc.tile_pool(name="tmp", bufs=3) as tpool,
            tc.tile_pool(name="psum", bufs=1, space="PSUM") as ppool,
        ):
            At = cpool.tile([P, FL], F32, tag="At")
            Bt = cpool.tile([P, FL], F32, tag="Bt")
            S1 = cpool.tile([P, P], F32, tag="S1")       # shifted identity
            E127 = cpool.tile([1, P], F32, tag="E127")   # one-hot row
            Ttl = cpool.tile([1, 3 * FL], F32, tag="Ttl")
            Srt = mpool.tile([P, G * FL], F32, tag="Sr")
            Sit = mpool.tile([P, G * FL], F32, tag="Si")
            Ft = mpool.tile([P, G * FL], F32, tag="F")
            Ot = mpool.tile([P, G * FS], F32, tag="O")
            Hp = ppool.tile([P, 2048], F32, tag="Hp")    # halo via PE, 4 banks

            # constants: coefficient broadcast (step-0 DMA) on the tensor
            # engine's queue so it doesn't delay the input stream on sync
            nc.scalar.dma_start(out=At[:, :], in_=ca_d[None, :].broadcast_to([P, FL]))
            nc.scalar.dma_start(out=Bt[:, :], in_=cb_d[None, :].broadcast_to([P, FL]))
            nc.scalar.dma_start(out=S1[:, :], in_=sw2[0:P, :])
            nc.scalar.dma_start(out=E127[:, :], in_=sw2[P:P + 1, :])
            nc.scalar.dma_start(out=Ttl[:, :], in_=ft_d[None, :])

            # stream input + elementwise F, one e-slice at a time (the DVE is
            # the serial bottleneck; the DMA stream stays ahead of it)
            for e in range(G):
                sl = slice(e * FL, (e + 1) * FL)
                nc.sync.dma_start(out=Srt[:, sl], in_=sr3[:, e, :])
                nc.sync.dma_start(out=Sit[:, sl], in_=si3[:, e, :])
                t = tpool.tile([P, FL], F32, tag="t")
                nc.vector.tensor_tensor(out=t[:, :], in0=Sit[:, sl], in1=Bt[:, :], op=ALU.mult)
                nc.vector.tensor_tensor(out=Ft[:, sl], in0=Srt[:, sl], in1=At[:, :], op=ALU.mult)
                nc.vector.tensor_tensor(out=Ft[:, sl], in0=Ft[:, sl], in1=t[:, :], op=ALU.add)

            Fv = Ft[:, :].rearrange("p (g k) -> p g k", g=G)
            Tv = Ttl[:, :].rearrange("p (g k) -> p g k", g=3)
            Ov = Ot[:, :].rearrange("p (g r) -> p g r", g=G)

            # halo by PE shift: Hp[q, :] = F[q+1, sel] (+ tail for q=127)
            # layout: [0:768]   = frames 0..2 chunk0   (read by d=3)
            #         [1024:1536] = frames 0..1 chunk1 (read by d=2)
            #         [1536:1792] = frame 0 chunk2     (read by d=1)
            mm = [
                (slice(0, 512),      (slice(0, 2), slice(0, FS))),        # f01 c0
                (slice(512, 768),    (slice(2, 3), slice(0, FS))),        # f2  c0
                (slice(1024, 1536),  (slice(0, 2), slice(FS, 2 * FS))),   # f01 c1
                (slice(1536, 1792),  (slice(0, 1), slice(2 * FS, 3 * FS))),  # f0 c2
            ]
            for osl, (gsl, ksl) in mm:
                nc.tensor.matmul(Hp[:, osl], S1[:, :], Fv[:, gsl, ksl],
                                 start=True, stop=False)
                nc.tensor.matmul(Hp[:, osl], E127[:, :], Tv[:, gsl, ksl],
                                 start=False, stop=True)

            # overlap-add in two halves so half A streams out early.
            # out[p, b_e] = sum_d F[p, b_e+d, chunk(3-d)], wrap terms from Hp
            # half A: b_e 0..3 (needs F e <= 6, no wrap)
            nc.scalar.copy(out=Ov[:, 0:4, :], in_=Fv[:, 0:4, 3 * FS:4 * FS])
            for d in (1, 2, 3):
                c = 3 - d
                csl = slice(c * FS, (c + 1) * FS)
                nc.vector.tensor_tensor(
                    out=Ov[:, 0:4, :], in0=Ov[:, 0:4, :],
                    in1=Fv[:, d:4 + d, csl], op=ALU.add)
            nc.sync.dma_start(out=out2[:, 0:4 * FS], in_=Ot[:, 0:4 * FS])

            # half B: b_e 4..7 (wrap terms read PSUM)
            nc.scalar.copy(out=Ov[:, 4:8, :], in_=Fv[:, 4:8, 3 * FS:4 * FS])
            hp_sl = {1: slice(1536, 1792), 2: slice(1024, 1536), 3: slice(0, 768)}
            for d in (1, 2, 3):
                c = 3 - d
                csl = slice(c * FS, (c + 1) * FS)
                nc.vector.tensor_tensor(
                    out=Ov[:, 4:8 - d, :], in0=Ov[:, 4:8 - d, :],
                    in1=Fv[:, 4 + d:8, csl], op=ALU.add)
                # wrap blocks b_e = 8-d..7 <- Hp (flat slices so shapes match)
                osl = slice((8 - d) * FS, 8 * FS)
                nc.vector.tensor_tensor(
                    out=Ot[:, osl], in0=Ot[:, osl],
                    in1=Hp[:, hp_sl[d]], op=ALU.add)
            nc.scalar.dma_start(out=out2[:, 4 * FS:], in_=Ot[:, 4 * FS:])
    nc.compile()
    return nc


_cache = {}


def _get_nc():
    if "nc" not in _cache:
        _cache["nc"] = build_nc()
    return _cache["nc"]


def make_in_maps(S_real, S_imag):
    a, b = _coeffs()
    pad = np.zeros((3, FL), dtype=np.float32)
    sr_pad = np.concatenate([pad, S_real], axis=0)
    si_pad = np.concatenate([pad, S_imag], axis=0)
    shiftw = _shift_weights().reshape(-1)
    in_maps = []
    for m in range(NCORES):
        r0 = m * FPC
        hi = m * FPC + FPC - 3
        # host-computed F for this core's last 3 own frames (feeds partition
        # 127's halo)
        ftl = (S_real[hi:hi + 3] * a[None, :] + S_imag[hi:hi + 3] * b[None, :])
        in_maps.append({
            "s_real": np.ascontiguousarray(sr_pad[r0:r0 + ROWS]),
            "s_imag": np.ascontiguousarray(si_pad[r0:r0 + ROWS]),
            "coef_a": a,
            "coef_b": b,
            "f_tail": np.ascontiguousarray(ftl.reshape(-1)),
            "shiftw": shiftw,
        })
    return in_maps


def assemble_output(S_real, S_imag, segs):
    a, b = _coeffs()
    out = np.zeros(OUT_LEN, dtype=np.float32)
    for m in range(NCORES):
        out[m * FPC * FS:(m + 1) * FPC * FS] = segs[m]

    # global tail: blocks 8192..8194 from the last 3 frames
    hf = (S_real[NF - 3:] * a[None, :] + S_imag[NF - 3:] * b[None, :])
    for t in range(3):
        i = NF - 3 + t
        for j in range(3 - t, 4):
            blk = i + j
            out[blk * FS:(blk + 1) * FS] += hf[t, j * FS:(j + 1) * FS]

    if "corr" not in _cache:
        _cache["corr"] = _window_correction()
    corr = _cache["corr"]
    return out / np.maximum(corr, np.float32(1e-8))


def kernel(S_real, S_imag):
    S_real = np.asarray(S_real, dtype=np.float32)
    S_imag = np.asarray(S_imag, dtype=np.float32)
    in_maps = make_in_maps(S_real, S_imag)
    nc = _get_nc()
    res = run_bass_kernel_spmd(nc, in_maps, list(range(NCORES)))
    segs = [res.results[m]["out_seg"] for m in range(NCORES)]
    return assemble_output(S_real, S_imag, segs)



# revision 7
# speedup vs baseline: 1.6905x; 1.6905x over previous
"""Trainium2 Bass kernel: custom inverse STFT (degenerate per-bin rotation +
Hann window + overlap-add + window correction).

Math (matching the reference):
    F[i,k]  = S_real[i,k]*A[k] + S_imag[i,k]*B[k]
      A[k]  = w[k]*(cos(th)-sin(th))/n,  B[k] = -w[k]*(cos(th)+sin(th))/n
    out[t]  = sum_i F[i, t-256*i] / max(corr[t], 1e-8)

Implementation (fp16 datapath, f32 accumulation):
  - Inputs are cast to fp16 on the host (halves HBM traffic; the 2e-2 rel-err
    budget dwarfs fp16 rounding).  Coefficients are scaled by 2^8 so products
    stay in fp16's normal range; the host divides the scale back out.
  - Sharding: 8192 frames -> 8 cores x 1024 frames (+3 left-halo frames).
  - On-chip layout per core: 9 slices of 128 consecutive frames starting at
    multiples of 125 (slices overlap by 3 frames), frame = partition.  Each
    slice yields 125 output blocks whose 4 overlap-add contributions all live
    on partitions of the SAME slice -> the whole overlap-add (including the
    t1+t2 sum) runs on the TensorEngine as shifted-identity matmuls
    accumulating exactly in f32 PSUM.  The DVE only does the two fp16
    products (2x packed mode); the Scalar engine stages PSUM->SBUF.
  - The 6 edge blocks (corr < 2, where the window-correction division
    amplifies fp16 underflow) are recomputed exactly in f32 on the host,
    as is the 768-sample global tail.
"""

import numpy as np

import concourse.bass as bass
import concourse.bacc as bacc
import concourse.mybir as mybir
import concourse.tile as tile
from concourse.bass_utils import run_bass_kernel_spmd

F16 = mybir.dt.float16
F32 = mybir.dt.float32
ALU = mybir.AluOpType

P = 128            # SBUF partitions
FL = 1024          # frame length (== fft length)
FS = 256           # frame step
NF = 8192          # total frames
NCORES = 8
FPC = NF // NCORES          # frames owned per core (1024)
ROWS = FPC + 3              # input rows per core (3 left-halo frames)
SL = 125                    # slice stride in frames (overlap of 3)
NS = 9                      # slices (8 full x 125 blocks + 1 partial x 24)
OUT_LEN = FS * (NF - 1) + FL
SCALE = np.float32(256.0)   # exact power-of-2 coefficient pre-scale


def _window32():
    # bit-matches the reference's f32 window computation
    k = np.arange(FL, dtype=np.float32)
    th = np.float32(2.0 * np.pi) * k / np.float32(FL)
    return (np.float32(0.5) - np.float32(0.5) * np.cos(th)).astype(np.float32)


def _coeffs32():
    k = np.arange(FL, dtype=np.float64)
    th = 2.0 * np.pi * k / FL
    w = _window32().astype(np.float64)
    a = (w * (np.cos(th) - np.sin(th)) / FL).astype(np.float32)
    b = (-w * (np.cos(th) + np.sin(th)) / FL).astype(np.float32)
    return a, b


def _window_correction():
    w = _window32()
    corr = np.zeros(OUT_LEN, dtype=np.float32)
    for j in range(4):
        view = corr[j * FS:j * FS + NF * FS].reshape(NF, FS)
        view += w[j * FS:(j + 1) * FS][None, :]
    return corr


def build_nc():
    nc = bacc.Bacc(trn_type="TRN2", target_bir_lowering=False, debug=False)
    sr_d = nc.dram_tensor("s_real", [ROWS, FL], F16, kind="ExternalInput").ap()
    si_d = nc.dram_tensor("s_imag", [ROWS, FL], F16, kind="ExternalInput").ap()
    cf_d = nc.dram_tensor("coefs", [2 * FL + P], F16, kind="ExternalInput").ap()
    id_d = nc.dram_tensor("ident", [P, P], F16, kind="ExternalInput").ap()
    out_d = nc.dram_tensor("out_seg", [FPC * FS], F16, kind="ExternalOutput").ap()

    def win_src(d, s0, nsl):
        # DRAM view [128, nsl, 1024]: partition p, sub-slice j reads row
        # (s0+j)*125 + p.  Consecutive slices overlap by 3 rows, so this is
        # built as a raw AP (rearrange can't express overlap).
        return bass.AP(d.tensor, s0 * SL * FL, [(FL, P), (SL * FL, nsl), (1, FL)])

    with tile.TileContext(nc) as tc:
        with (
            tc.tile_pool(name="const", bufs=1) as cpool,
            tc.tile_pool(name="main", bufs=1) as mpool,
            tc.tile_pool(name="psum", bufs=1, space="PSUM") as ppool,
        ):
            Crow = cpool.tile([1, 2 * FL + P], F16, tag="Crow")  # A*2^8|B*2^8|ones
            Id = cpool.tile([P, P], F16, tag="Id")
            At = cpool.tile([P, FL], F16, tag="At")
            Bt = cpool.tile([P, FL], F16, tag="Bt")
            Srt = mpool.tile([P, NS * FL], F16, tag="Sr")
            Sit = mpool.tile([P, NS * FL], F16, tag="Si")
            T1 = mpool.tile([P, NS * FL], F16, tag="T1")
            T2 = mpool.tile([P, NS * FL], F16, tag="T2")
            Ot = mpool.tile([P, NS * FS], F16, tag="Ot")
            ABp = ppool.tile([P, FL], F32, tag="ABp")
            Op = ppool.tile([P, NS * FS], F32, tag="Op")

            # constants on the ACT queue (ahead of the Si input stream)
            nc.scalar.dma_start(out=Crow[:, :], in_=cf_d[None, :])
            nc.scalar.dma_start(out=Id[:, :], in_=id_d[:, :])

            # slice 8 is partial (27 of 128 rows DMA'd); zero it so the
            # 0-weighted matmul lanes can't turn SBUF garbage into NaN
            nc.gpsimd.memset(Srt[:, 8 * FL:9 * FL], 0.0)
            nc.gpsimd.memset(Sit[:, 8 * FL:9 * FL], 0.0)

            # input stream: 512KB pair DMAs, Sr on sync queue, Si on ACT queue
            for s0 in (0, 2, 4, 6):
                dst = Srt[:, s0 * FL:(s0 + 2) * FL].rearrange("p (s k) -> p s k", s=2)
                nc.sync.dma_start(out=dst, in_=win_src(sr_d, s0, 2))
                dst = Sit[:, s0 * FL:(s0 + 2) * FL].rearrange("p (s k) -> p s k", s=2)
                nc.scalar.dma_start(out=dst, in_=win_src(si_d, s0, 2))
            nc.sync.dma_start(out=Srt[0:27, 8 * FL:9 * FL], in_=sr_d[8 * SL:ROWS, :])
            nc.scalar.dma_start(out=Sit[0:27, 8 * FL:9 * FL], in_=si_d[8 * SL:ROWS, :])

            # broadcast A,B to all partitions via K=1 matmul (ones @ row)
            ones = Crow[0:1, 2 * FL:2 * FL + P]
            for h in range(2):
                nc.tensor.matmul(ABp[:, h * 512:(h + 1) * 512], ones,
                                 Crow[0:1, h * 512:(h + 1) * 512],
                                 start=True, stop=True)
            nc.vector.tensor_copy(out=At[:, :], in_=ABp[:, :])
            for h in range(2):
                nc.tensor.matmul(ABp[:, h * 512:(h + 1) * 512], ones,
                                 Crow[0:1, FL + h * 512:FL + (h + 1) * 512],
                                 start=True, stop=True)
            nc.vector.tensor_copy(out=Bt[:, :], in_=ABp[:, :])

            # fp16 products on the DVE (2x packed mode), one slice at a time
            for s in range(NS):
                sl = slice(s * FL, (s + 1) * FL)
                nc.vector.tensor_tensor(out=T1[:, sl], in0=Srt[:, sl],
                                        in1=At[:, :], op=ALU.mult)
                nc.vector.tensor_tensor(out=T2[:, sl], in0=Sit[:, sl],
                                        in1=Bt[:, :], op=ALU.mult)

            # overlap-add on the PE: slice s, output block m (0..124), chunk c
            # reads frame q = m+3-c of the same slice -> lhsT = Id[:, 3-c:128-c]
            T1v = T1[:, :].rearrange("p (s k) -> p s k", s=NS)
            T2v = T2[:, :].rearrange("p (s k) -> p s k", s=NS)
            pairs = [(0, 2), (2, 2), (4, 2), (6, 2), (8, 1)]
            for s0, nsl in pairs:
                osl = slice(s0 * FS, (s0 + nsl) * FS)
                for c in range(4):
                    ksl = slice(c * FS, (c + 1) * FS)
                    w = Id[:, 3 - c:3 - c + SL]
                    nc.tensor.matmul(Op[0:SL, osl], w, T1v[:, s0:s0 + nsl, ksl],
                                     start=(c == 0), stop=False)
                    nc.tensor.matmul(Op[0:SL, osl], w, T2v[:, s0:s0 + nsl, ksl],
                                     start=False, stop=(c == 3))
                # PSUM -> SBUF fp16 staging on the ACT engine
                nc.scalar.copy(out=Ot[0:SL, osl], in_=Op[0:SL, osl])

            # store: block s*125+m -> out[(s*125+m)*256 : +256]
            d1 = out_d[0:4 * SL * FS].rearrange("(s p r) -> p s r", s=4, p=SL)
            nc.sync.dma_start(out=d1, in_=Ot[0:SL, 0:4 * FS].rearrange(
                "p (s r) -> p s r", s=4))
            d2 = out_d[4 * SL * FS:8 * SL * FS].rearrange("(s p r) -> p s r", s=4, p=SL)
            nc.sync.dma_start(out=d2, in_=Ot[0:SL, 4 * FS:8 * FS].rearrange(
                "p (s r) -> p s r", s=4))
            d3 = out_d[8 * SL * FS:FPC * FS].rearrange("(p r) -> p r", p=24)
            nc.sync.dma_start(out=d3, in_=Ot[0:24, 8 * FS:9 * FS])
    nc.compile()
    return nc


_cache = {}


def _get_nc():
    if "nc" not in _cache:
        _cache["nc"] = build_nc()
    return _cache["nc"]


def make_in_maps(S_real, S_imag):
    a32, b32 = _coeffs32()
    coefs = np.zeros(2 * FL + P, dtype=np.float16)
    coefs[0:FL] = (a32 * SCALE).astype(np.float16)
    coefs[FL:2 * FL] = (b32 * SCALE).astype(np.float16)
    coefs[2 * FL:] = np.float16(1.0)
    ident = np.eye(P, dtype=np.float16)

    pad = np.zeros((3, FL), dtype=np.float16)
    sr16 = np.concatenate([pad, S_real.astype(np.float16)], axis=0)
    si16 = np.concatenate([pad, S_imag.astype(np.float16)], axis=0)

    in_maps = []
    for m in range(NCORES):
        r0 = m * FPC
        in_maps.append({
            "s_real": np.ascontiguousarray(sr16[r0:r0 + ROWS]),
            "s_imag": np.ascontiguousarray(si16[r0:r0 + ROWS]),
            "coefs": coefs,
            "ident": ident,
        })
    return in_maps


def assemble_output(S_real, S_imag, segs):
    a32, b32 = _coeffs32()
    full = np.empty(OUT_LEN, dtype=np.float32)
    inv_scale = np.float32(1.0) / SCALE
    for m in range(NCORES):
        full[m * FPC * FS:(m + 1) * FPC * FS] = segs[m].astype(np.float32) * inv_scale

    # exact f32 recompute of the 6 edge blocks (corr < 2 there: the final
    # division amplifies fp16 error by up to ~1e5) and the global tail
    Fh = S_real[0:3] * a32[None, :] + S_imag[0:3] * b32[None, :]
    full[0:FS] = Fh[0, 0:FS]
    full[FS:2 * FS] = Fh[0, FS:2 * FS] + Fh[1, 0:FS]
    full[2 * FS:3 * FS] = Fh[0, 2 * FS:3 * FS] + Fh[1, FS:2 * FS] + Fh[2, 0:FS]
    Ft = S_real[NF - 3:] * a32[None, :] + S_imag[NF - 3:] * b32[None, :]
    full[NF * FS:NF * FS + FS] = Ft[0, 3 * FS:] + Ft[1, 2 * FS:3 * FS] + Ft[2, FS:2 * FS]
    full[NF * FS + FS:NF * FS + 2 * FS] = Ft[1, 3 * FS:] + Ft[2, 2 * FS:3 * FS]
    full[NF * FS + 2 * FS:] = Ft[2, 3 * FS:]

    if "corr" not in _cache:
        _cache["corr"] = np.maximum(_window_correction(), np.float32(1e-8))
    return full / _cache["corr"]


def kernel(S_real, S_imag):
    S_real = np.asarray(S_real, dtype=np.float32)
    S_imag = np.asarray(S_imag, dtype=np.float32)
    in_maps = make_in_maps(S_real, S_imag)
    nc = _get_nc()
    res = run_bass_kernel_spmd(nc, in_maps, list(range(NCORES)))
    segs = [res.results[m]["out_seg"] for m in range(NCORES)]
    return assemble_output(S_real, S_imag, segs)
